# revision 10
# baseline (speedup 1.0000x reference)
"""Trainium2 Bass kernel for the n-gram induction-head + dual-linear problem.

  y = induction_head(input_ids, x) @ W0.T + b0 + x @ W1.T + b1
  (B=4, S=2048, D=1024, ngram=2, shift_step=1)

Mask algebra (equivalent to the reference):
  maskT[j, i] = (C[i] == A[j]) & (i > j)
    A[j] = 512*ids[j-2] + ids[j-1]   (sentinels ids[-2]=-11, ids[-1]=-13)
    C[i] = 512*ids[i-1] + ids[i]
  h0[i,:] = (sum_j maskT[j,i] * x[j,:]) / max(sum_j maskT[j,i], 1)

Distribution: 8 cores = 4 batches x 2 row-groups. Group 0 takes rows
[1536:2048]+[0:512], group 1 takes [1024:1536]+[512:1024] of its batch, which
balances the causal-triangle matmul work exactly (20 live j-chunk passes per
core). The whole computation runs in a transposed dataflow (h0^T, y^T) so no
on-chip transposes are needed; the host un-transposes the per-core outputs.

The single SPMD program is identical on all cores; all per-core variation
(row ranges, ids, triangle thresholds) enters via input data.
"""

import os
from contextlib import ExitStack

import numpy as np

import concourse.bass as bass  # noqa: F401  (engine types referenced via nc)
import concourse.mybir as mybir
import concourse.tile as tile
from concourse import bacc
from concourse.bass_utils import run_bass_kernel_spmd

F32 = mybir.dt.float32
S = 2048
D = 1024
NSLOTS = [16, 8]
ROWS = {
    0: [(1536, 2048), (0, 512)],
    1: [(1024, 1536), (512, 1024)],
}


def _tri_slots(t):
    return range(8, 16) if t == 0 else range(0, 8)


def _build_kernel(tc, io, mm_dt):
    nc = tc.nc
    op = mybir.AluOpType
    xf, xt, w0t, w1t, yt = (
        io["xf"], io["xt"], io["w0t"], io["w1t"], io["yt"],
    )

    with ExitStack() as ctx:
        consts = ctx.enter_context(tc.tile_pool(name="consts", bufs=1))
        maskp = ctx.enter_context(tc.tile_pool(name="maskp", bufs=24))
        trip = ctx.enter_context(tc.tile_pool(name="trip", bufs=2))
        bcast = ctx.enter_context(tc.tile_pool(name="bcast", bufs=2))
        h0p = ctx.enter_context(tc.tile_pool(name="h0p", bufs=2))
        xtp = ctx.enter_context(tc.tile_pool(name="xtp", bufs=2))
        youtp = ctx.enter_context(tc.tile_pool(name="youtp", bufs=4))
        psum = ctx.enter_context(tc.tile_pool(name="psum", bufs=8, space="PSUM"))

        # ---- tiles ----
        ones = consts.tile([128, 128], mm_dt)
        smalls = consts.tile([128, 56], F32)
        d0 = smalls[:, 0:16]
        d1 = smalls[:, 16:32]
        jv = smalls[:, 32:48]
        bcols = smalls[:, 48:56]
        a16 = consts.tile([128, 16], F32)
        xall = consts.tile([128, 16 * 1024], mm_dt)
        w0 = consts.tile([128, 8 * 1024], mm_dt)
        w1 = consts.tile([128, 8 * 1024], mm_dt)
        cbs, ivbs, e0bs, mrows = {}, {}, {}, {}
        for t in range(2):
            mr = bcast.tile([128, 3 * 512], F32, tag="mr", name=f"mr{t}")
            mrows[t] = mr
            cbs[t] = mr[:, 0:512]
            e0bs[t] = mr[:, 512:1024]
            ivbs[t] = mr[:, 1024:1536]

        # ---- DMA issue order (sync HWDGE queue): critical path first ----
        nc.sync.dma_start(smalls[:], io["smalls"][:])
        nc.sync.dma_start(mrows[0][:], io["mrowsd"][0])
        nc.sync.dma_start(xall[:, 0:1024], xf[0])
        nc.sync.dma_start(xall[:, 1024:2048], xf[1])
        for s_ in range(2, 16):
            nc.sync.dma_start(xall[:, s_ * 1024 : (s_ + 1) * 1024], xf[s_])
        nc.sync.dma_start(mrows[1][:], io["mrowsd"][1])
        nc.sync.dma_start(ones[:], io["onesd"][:])
        for kc in range(8):
            nc.sync.dma_start(w0[:, kc * 1024 : (kc + 1) * 1024], w0t[kc])
            nc.sync.dma_start(w1[:, kc * 1024 : (kc + 1) * 1024], w1t[kc])

        # ---- compute ----
        nc.vector.tensor_scalar(a16[:], d0[:], 512.0, None, op.mult)
        nc.vector.tensor_tensor(a16[:], a16[:], d1[:], op.add)

        masks = {0: [], 1: []}
        h0s = {}

        def build_masks(t):
            cb, ivb = cbs[t], ivbs[t]
            nc.vector.tensor_scalar(cb[:], cb[:], 512.0, None, op.mult)
            nc.vector.tensor_tensor(cb[:], cb[:], e0bs[t][:], op.add)
            tri = set(_tri_slots(t))
            for s in range(NSLOTS[t]):
                m = maskp.tile([128, 512], mm_dt, tag="m", name=f"m{t}_{s}")
                if s in tri:
                    tr = trip.tile([128, 512], mm_dt, tag="tr", name=f"tr{t}_{s}")
                    nc.vector.tensor_scalar(
                        tr[:], ivb[:], jv[:, s : s + 1], None, op.is_gt
                    )
                    nc.vector.scalar_tensor_tensor(
                        m[:], cb[:], a16[:, s : s + 1], tr[:], op.is_equal, op.mult
                    )
                else:
                    nc.vector.tensor_scalar(
                        m[:], cb[:], a16[:, s : s + 1], None, op.is_equal
                    )
                masks[t].append(m)

        def attention(t):
            att = [
                psum.tile([128, 512], F32, tag="ps", name=f"att{t}_{dc}")
                for dc in range(8)
            ]
            for s in range(NSLOTS[t]):
                for dc in range(8):
                    nc.tensor.matmul(
                        att[dc][:],
                        xall[:, s * 1024 + dc * 128 : s * 1024 + dc * 128 + 128],
                        masks[t][s][:],
                        start=(s == 0),
                        stop=(s == NSLOTS[t] - 1),
                    )
            # raw (unnormalized) h0 evicted on the Scalar engine
            h0 = h0p.tile([128, 8 * 512], mm_dt, tag="h0", name=f"h0_{t}")
            for dc in range(8):
                nc.scalar.copy(h0[:, dc * 512 : (dc + 1) * 512], att[dc][:])
            h0s[t] = h0

        def denom(t):
            dn = psum.tile([128, 512], F32, tag="ps", name=f"dn{t}")
            for s in range(NSLOTS[t]):
                nc.tensor.matmul(
                    dn[:], ones[:], masks[t][s][:],
                    start=(s == 0), stop=(s == NSLOTS[t] - 1),
                )
            dcl = bcast.tile([128, 512], F32, tag="dcl", name=f"dcl{t}")
            nc.vector.tensor_scalar(dcl[:], dn[:], 1.0, None, op.max)
            rec = bcast.tile([128, 512], F32, tag="rec", name=f"rec{t}")
            nc.vector.reciprocal(rec[:], dcl[:])
            h0 = h0s[t]
            for dc in range(8):
                nc.vector.tensor_tensor(
                    h0[:, dc * 512 : (dc + 1) * 512],
                    h0[:, dc * 512 : (dc + 1) * 512],
                    rec[:],
                    op.mult,
                )

        def linears(t):
            h0 = h0s[t]
            xts = xtp.tile([128, 8 * 512], mm_dt, tag="xts", name=f"xts{t}")
            for kc in range(8):
                nc.sync.dma_start(xts[:, kc * 512 : (kc + 1) * 512], xt[t, kc])
            for oc in range(8):
                yps = psum.tile([128, 512], F32, tag="ps", name=f"yps{t}_{oc}")
                for kc in range(8):
                    nc.tensor.matmul(
                        yps[:],
                        w0[:, kc * 1024 + oc * 128 : kc * 1024 + oc * 128 + 128],
                        h0[:, kc * 512 : (kc + 1) * 512],
                        start=(kc == 0),
                        stop=False,
                    )
                for kc in range(8):
                    nc.tensor.matmul(
                        yps[:],
                        w1[:, kc * 1024 + oc * 128 : kc * 1024 + oc * 128 + 128],
                        xts[:, kc * 512 : (kc + 1) * 512],
                        start=False,
                        stop=(kc == 7),
                    )
                ysb = youtp.tile([128, 512], F32, tag="ysb", name=f"ysb{t}_{oc}")
                nc.scalar.activation(
                    ysb[:],
                    yps[:],
                    mybir.ActivationFunctionType.Identity,
                    bias=bcols[:, oc : oc + 1],
                    scale=1.0,
                )
                nc.sync.dma_start(yt[t, oc], ysb[:])

        build_masks(0)
        attention(0)
        build_masks(1)
        denom(0)
        attention(1)
        denom(1)
        linears(0)
        linears(1)


def _build_nc(mm_dt):
    nc = bacc.Bacc("TRN2", target_bir_lowering=False, debug=False)
    io = {
        "xf": nc.dram_tensor("xf", [16, 128, 1024], mm_dt, kind="ExternalInput"),
        "onesd": nc.dram_tensor("onesd", [128, 128], mm_dt, kind="ExternalInput"),
        "xt": nc.dram_tensor("xt", [2, 8, 128, 512], mm_dt, kind="ExternalInput"),
        "w0t": nc.dram_tensor("w0t", [8, 128, 1024], mm_dt, kind="ExternalInput"),
        "w1t": nc.dram_tensor("w1t", [8, 128, 1024], mm_dt, kind="ExternalInput"),
        "smalls": nc.dram_tensor("smalls", [128, 56], F32, kind="ExternalInput"),
        "mrowsd": nc.dram_tensor(
            "mrowsd", [2, 128, 3 * 512], F32, kind="ExternalInput"
        ),
        "yt": nc.dram_tensor("yt", [2, 8, 128, 512], F32, kind="ExternalOutput"),
    }
    with tile.TileContext(nc) as tc:
        _build_kernel(tc, {k: v.ap() for k, v in io.items()}, mm_dt)
    nc.compile()
    return nc


def _make_core_inputs(c, x, ids_i, W0, b0, W1, b1, np_mm=np.float32):
    b, g = c // 2, c % 2
    xb = np.ascontiguousarray(x[b], dtype=np.float32)
    idb = ids_i[b].astype(np.float32)
    P = np.empty(S + 4, np.float32)
    P[0], P[1] = -11.0, -13.0
    P[2 : S + 2] = idb
    P[S + 2 :] = -15.0
    rows = ROWS[g]

    xt = np.empty((2, 8, 128, 512), np.float32)
    mrowsd = np.empty((2, 128, 3 * 512), np.float32)
    for t, (r0, r1) in enumerate(rows):
        xt[t] = xb[r0:r1, :].T.reshape(8, 128, 512)
        mrowsd[t, :, 0:512] = P[r0 + 1 : r0 + 513][None, :]  # cb raw = ids[i-1]
        mrowsd[t, :, 512:1024] = P[r0 + 2 : r0 + 514][None, :]  # e0b = ids[i]
        mrowsd[t, :, 1024:1536] = np.arange(r0, r1, dtype=np.float32)[None, :]

    smalls = np.empty((128, 56), np.float32)
    smalls[:, 0:16] = P[0:2048].reshape(16, 128).T
    smalls[:, 16:32] = P[1:2049].reshape(16, 128).T
    smalls[:, 32:48] = (
        np.arange(16)[None, :] * 128 + np.arange(128)[:, None]
    ).astype(np.float32)
    smalls[:, 48:56] = (
        (b0.astype(np.float32) + b1.astype(np.float32)).reshape(8, 128).T
    )
    return {
        "xf": xb.reshape(16, 128, 1024).astype(np_mm),
        "onesd": np.ones((128, 128), np_mm),
        "xt": xt.astype(np_mm),
        "w0t": np.ascontiguousarray(W0.T.astype(np.float32)).reshape(8, 128, 1024).astype(np_mm),
        "w1t": np.ascontiguousarray(W1.T.astype(np.float32)).reshape(8, 128, 1024).astype(np_mm),
        "smalls": smalls,
        "mrowsd": mrowsd,
    }


_NC_CACHE = {}


def kernel(**inputs):
    x = np.asarray(inputs["x"])
    ids = np.asarray(inputs["input_ids"])
    W0 = np.asarray(inputs["W0"])
    b0 = np.asarray(inputs["b0"])
    W1 = np.asarray(inputs["W1"])
    b1 = np.asarray(inputs["b1"])

    sel = os.environ.get("NGRAM_MM_DT", "bf16")
    mm_dt = {
        "f32": mybir.dt.float32,
        "f32r": mybir.dt.float32r,
        "bf16": mybir.dt.bfloat16,
    }[sel]
    key = str(mm_dt)
    if key not in _NC_CACHE:
        _NC_CACHE[key] = _build_nc(mm_dt)
    nc = _NC_CACHE[key]

    import ml_dtypes

    np_mm = (
        ml_dtypes.bfloat16 if mm_dt == mybir.dt.bfloat16 else np.float32
    )
    in_maps = [
        _make_core_inputs(c, x, ids, W0, b0, W1, b1, np_mm) for c in range(8)
    ]
    core_ids = list(range(8))
    trace = os.environ.get("NGRAM_TRACE", "") == "1"
    res = run_bass_kernel_spmd(nc, in_maps, core_ids, trace=trace)
    if trace and res.exec_time_ns is not None:
        print(f"HW exec time: {res.exec_time_ns} ns")
        kernel.last_exec_time_ns = res.exec_time_ns

    y = np.empty((4, S, D), np.float32)
    for c in range(8):
        b, g = c // 2, c % 2
        ytc = res.results[c]["yt"]
        for t, (r0, r1) in enumerate(ROWS[g]):
            y[b, r0:r1, :] = ytc[t].reshape(1024, 512).T
    return y


# revision 11
# speedup vs baseline: 1.0075x; 1.0075x over previous
"""Trainium2 Bass kernel for the n-gram induction-head + dual-linear problem.

  y = induction_head(input_ids, x) @ W0.T + b0 + x @ W1.T + b1
  (B=4, S=2048, D=1024, ngram=2, shift_step=1)

Mask algebra (equivalent to the reference):
  maskT[j, i] = (C[i] == A[j]) & (i > j)
    A[j] = 512*ids[j-2] + ids[j-1]   (sentinels ids[-2]=-11, ids[-1]=-13)
    C[i] = 512*ids[i-1] + ids[i]
  h0[i,:] = (sum_j maskT[j,i] * x[j,:]) / max(sum_j maskT[j,i], 1)

Distribution: 8 cores = 4 batches x 2 row-groups. Group 0 takes rows
[1536:2048]+[0:512], group 1 takes [1024:1536]+[512:1024] of its batch, which
balances the causal-triangle matmul work exactly (20 live j-chunk passes per
core). The whole computation runs in a transposed dataflow (h0^T, y^T) so no
on-chip transposes are needed; the host un-transposes the per-core outputs.

The single SPMD program is identical on all cores; all per-core variation
(row ranges, ids, triangle thresholds) enters via input data.
"""

import os
from contextlib import ExitStack

import numpy as np

import concourse.bass as bass  # noqa: F401  (engine types referenced via nc)
import concourse.mybir as mybir
import concourse.tile as tile
from concourse import bacc
from concourse.bass_utils import run_bass_kernel_spmd

F32 = mybir.dt.float32
S = 2048
D = 1024
NSLOTS = [16, 8]
ROWS = {
    0: [(1536, 2048), (0, 512)],
    1: [(1024, 1536), (512, 1024)],
}


def _tri_slots(t):
    return range(8, 16) if t == 0 else range(0, 8)


def _build_kernel(tc, io, mm_dt):
    nc = tc.nc
    op = mybir.AluOpType
    xf, xt, w0t, w1t, yt = (
        io["xf"], io["xt"], io["w0t"], io["w1t"], io["yt"],
    )

    with ExitStack() as ctx:
        consts = ctx.enter_context(tc.tile_pool(name="consts", bufs=1))
        maskp = ctx.enter_context(tc.tile_pool(name="maskp", bufs=24))
        trip = ctx.enter_context(tc.tile_pool(name="trip", bufs=2))
        bcast = ctx.enter_context(tc.tile_pool(name="bcast", bufs=2))
        h0p = ctx.enter_context(tc.tile_pool(name="h0p", bufs=2))
        xtp = ctx.enter_context(tc.tile_pool(name="xtp", bufs=2))
        youtp = ctx.enter_context(tc.tile_pool(name="youtp", bufs=4))
        psum = ctx.enter_context(tc.tile_pool(name="psum", bufs=8, space="PSUM"))

        # ---- tiles ----
        ones = consts.tile([128, 128], mm_dt)
        smalls = consts.tile([128, 56], F32)
        d0 = smalls[:, 0:16]
        d1 = smalls[:, 16:32]
        jv = smalls[:, 32:48]
        bcols = smalls[:, 48:56]
        a16 = consts.tile([128, 16], F32)
        xall = consts.tile([128, 16 * 1024], mm_dt)
        w0 = consts.tile([128, 8 * 1024], mm_dt)
        w1 = consts.tile([128, 8 * 1024], mm_dt)
        cbs, ivbs, e0bs, mrows = {}, {}, {}, {}
        for t in range(2):
            mr = bcast.tile([128, 3 * 512], F32, tag="mr", name=f"mr{t}")
            mrows[t] = mr
            cbs[t] = mr[:, 0:512]
            e0bs[t] = mr[:, 512:1024]
            ivbs[t] = mr[:, 1024:1536]

        # ---- DMA issue order (sync HWDGE queue): critical path first ----
        nc.sync.dma_start(smalls[:], io["smalls"][:])
        nc.sync.dma_start(mrows[0][:], io["mrowsd"][0])
        nc.sync.dma_start(xall[:, 0:1024], xf[0])
        nc.sync.dma_start(xall[:, 1024:2048], xf[1])
        for s_ in range(2, 16):
            nc.sync.dma_start(xall[:, s_ * 1024 : (s_ + 1) * 1024], xf[s_])
        nc.sync.dma_start(mrows[1][:], io["mrowsd"][1])
        nc.sync.dma_start(ones[:], io["onesd"][:])
        for kc in range(8):
            nc.sync.dma_start(w0[:, kc * 1024 : (kc + 1) * 1024], w0t[kc])
            nc.sync.dma_start(w1[:, kc * 1024 : (kc + 1) * 1024], w1t[kc])

        # ---- compute ----
        nc.vector.scalar_tensor_tensor(
            a16[:], d0[:], 512.0, d1[:], op.mult, op.add
        )

        masks = {0: [], 1: []}
        h0s = {}

        def build_masks(t):
            cb, ivb = cbs[t], ivbs[t]
            nc.vector.scalar_tensor_tensor(
                cb[:], cb[:], 512.0, e0bs[t][:], op.mult, op.add
            )
            tri = set(_tri_slots(t))
            for s in range(NSLOTS[t]):
                m = maskp.tile([128, 512], mm_dt, tag="m", name=f"m{t}_{s}")
                if s in tri:
                    tr = trip.tile([128, 512], mm_dt, tag="tr", name=f"tr{t}_{s}")
                    nc.vector.tensor_scalar(
                        tr[:], ivb[:], jv[:, s : s + 1], None, op.is_gt
                    )
                    nc.vector.scalar_tensor_tensor(
                        m[:], cb[:], a16[:, s : s + 1], tr[:], op.is_equal, op.mult
                    )
                else:
                    nc.vector.tensor_scalar(
                        m[:], cb[:], a16[:, s : s + 1], None, op.is_equal
                    )
                masks[t].append(m)

        def attention(t):
            att = [
                psum.tile([128, 512], F32, tag="ps", name=f"att{t}_{dc}")
                for dc in range(8)
            ]
            for s in range(NSLOTS[t]):
                for dc in range(8):
                    nc.tensor.matmul(
                        att[dc][:],
                        xall[:, s * 1024 + dc * 128 : s * 1024 + dc * 128 + 128],
                        masks[t][s][:],
                        start=(s == 0),
                        stop=(s == NSLOTS[t] - 1),
                    )
            # raw (unnormalized) h0 evicted on the Scalar engine
            h0 = h0p.tile([128, 8 * 512], mm_dt, tag="h0", name=f"h0_{t}")
            for dc in range(8):
                nc.scalar.copy(h0[:, dc * 512 : (dc + 1) * 512], att[dc][:])
            h0s[t] = h0

        def denom(t):
            dn = psum.tile([128, 512], F32, tag="ps", name=f"dn{t}")
            for s in range(NSLOTS[t]):
                nc.tensor.matmul(
                    dn[:], ones[:], masks[t][s][:],
                    start=(s == 0), stop=(s == NSLOTS[t] - 1),
                )
            dcl = bcast.tile([128, 512], F32, tag="dcl", name=f"dcl{t}")
            nc.vector.tensor_scalar(dcl[:], dn[:], 1.0, None, op.max)
            rec = bcast.tile([128, 512], F32, tag="rec", name=f"rec{t}")
            nc.vector.reciprocal(rec[:], dcl[:])
            h0 = h0s[t]
            for dc in range(8):
                nc.vector.tensor_tensor(
                    h0[:, dc * 512 : (dc + 1) * 512],
                    h0[:, dc * 512 : (dc + 1) * 512],
                    rec[:],
                    op.mult,
                )

        def linears(t):
            h0 = h0s[t]
            xts = xtp.tile([128, 8 * 512], mm_dt, tag="xts", name=f"xts{t}")
            for kc in range(8):
                nc.sync.dma_start(xts[:, kc * 512 : (kc + 1) * 512], xt[t, kc])
            for oc in range(8):
                yps = psum.tile([128, 512], F32, tag="ps", name=f"yps{t}_{oc}")
                for kc in range(8):
                    nc.tensor.matmul(
                        yps[:],
                        w0[:, kc * 1024 + oc * 128 : kc * 1024 + oc * 128 + 128],
                        h0[:, kc * 512 : (kc + 1) * 512],
                        start=(kc == 0),
                        stop=False,
                    )
                for kc in range(8):
                    nc.tensor.matmul(
                        yps[:],
                        w1[:, kc * 1024 + oc * 128 : kc * 1024 + oc * 128 + 128],
                        xts[:, kc * 512 : (kc + 1) * 512],
                        start=False,
                        stop=(kc == 7),
                    )
                ysb = youtp.tile([128, 512], F32, tag="ysb", name=f"ysb{t}_{oc}")
                nc.scalar.activation(
                    ysb[:],
                    yps[:],
                    mybir.ActivationFunctionType.Identity,
                    bias=bcols[:, oc : oc + 1],
                    scale=1.0,
                )
                nc.sync.dma_start(yt[t, oc], ysb[:])

        build_masks(0)
        attention(0)
        build_masks(1)
        denom(0)
        attention(1)
        denom(1)
        linears(0)
        linears(1)


def _build_nc(mm_dt):
    nc = bacc.Bacc("TRN2", target_bir_lowering=False, debug=False)
    io = {
        "xf": nc.dram_tensor("xf", [16, 128, 1024], mm_dt, kind="ExternalInput"),
        "onesd": nc.dram_tensor("onesd", [128, 128], mm_dt, kind="ExternalInput"),
        "xt": nc.dram_tensor("xt", [2, 8, 128, 512], mm_dt, kind="ExternalInput"),
        "w0t": nc.dram_tensor("w0t", [8, 128, 1024], mm_dt, kind="ExternalInput"),
        "w1t": nc.dram_tensor("w1t", [8, 128, 1024], mm_dt, kind="ExternalInput"),
        "smalls": nc.dram_tensor("smalls", [128, 56], F32, kind="ExternalInput"),
        "mrowsd": nc.dram_tensor(
            "mrowsd", [2, 128, 3 * 512], F32, kind="ExternalInput"
        ),
        "yt": nc.dram_tensor("yt", [2, 8, 128, 512], F32, kind="ExternalOutput"),
    }
    with tile.TileContext(nc) as tc:
        _build_kernel(tc, {k: v.ap() for k, v in io.items()}, mm_dt)
    nc.compile()
    return nc


def _make_core_inputs(c, x, ids_i, W0, b0, W1, b1, np_mm=np.float32):
    b, g = c // 2, c % 2
    xb = np.ascontiguousarray(x[b], dtype=np.float32)
    idb = ids_i[b].astype(np.float32)
    P = np.empty(S + 4, np.float32)
    P[0], P[1] = -11.0, -13.0
    P[2 : S + 2] = idb
    P[S + 2 :] = -15.0
    rows = ROWS[g]

    xt = np.empty((2, 8, 128, 512), np.float32)
    mrowsd = np.empty((2, 128, 3 * 512), np.float32)
    for t, (r0, r1) in enumerate(rows):
        xt[t] = xb[r0:r1, :].T.reshape(8, 128, 512)
        mrowsd[t, :, 0:512] = P[r0 + 1 : r0 + 513][None, :]  # cb raw = ids[i-1]
        mrowsd[t, :, 512:1024] = P[r0 + 2 : r0 + 514][None, :]  # e0b = ids[i]
        mrowsd[t, :, 1024:1536] = np.arange(r0, r1, dtype=np.float32)[None, :]

    smalls = np.empty((128, 56), np.float32)
    smalls[:, 0:16] = P[0:2048].reshape(16, 128).T
    smalls[:, 16:32] = P[1:2049].reshape(16, 128).T
    smalls[:, 32:48] = (
        np.arange(16)[None, :] * 128 + np.arange(128)[:, None]
    ).astype(np.float32)
    smalls[:, 48:56] = (
        (b0.astype(np.float32) + b1.astype(np.float32)).reshape(8, 128).T
    )
    return {
        "xf": xb.reshape(16, 128, 1024).astype(np_mm),
        "onesd": np.ones((128, 128), np_mm),
        "xt": xt.astype(np_mm),
        "w0t": np.ascontiguousarray(W0.T.astype(np.float32)).reshape(8, 128, 1024).astype(np_mm),
        "w1t": np.ascontiguousarray(W1.T.astype(np.float32)).reshape(8, 128, 1024).astype(np_mm),
        "smalls": smalls,
        "mrowsd": mrowsd,
    }


_NC_CACHE = {}


def kernel(**inputs):
    x = np.asarray(inputs["x"])
    ids = np.asarray(inputs["input_ids"])
    W0 = np.asarray(inputs["W0"])
    b0 = np.asarray(inputs["b0"])
    W1 = np.asarray(inputs["W1"])
    b1 = np.asarray(inputs["b1"])

    sel = os.environ.get("NGRAM_MM_DT", "bf16")
    mm_dt = {
        "f32": mybir.dt.float32,
        "f32r": mybir.dt.float32r,
        "bf16": mybir.dt.bfloat16,
    }[sel]
    key = str(mm_dt)
    if key not in _NC_CACHE:
        _NC_CACHE[key] = _build_nc(mm_dt)
    nc = _NC_CACHE[key]

    import ml_dtypes

    np_mm = (
        ml_dtypes.bfloat16 if mm_dt == mybir.dt.bfloat16 else np.float32
    )
    in_maps = [
        _make_core_inputs(c, x, ids, W0, b0, W1, b1, np_mm) for c in range(8)
    ]
    core_ids = list(range(8))
    trace = os.environ.get("NGRAM_TRACE", "") == "1"
    res = run_bass_kernel_spmd(nc, in_maps, core_ids, trace=trace)
    if trace and res.exec_time_ns is not None:
        print(f"HW exec time: {res.exec_time_ns} ns")
        kernel.last_exec_time_ns = res.exec_time_ns

    y = np.empty((4, S, D), np.float32)
    for c in range(8):
        b, g = c // 2, c % 2
        ytc = res.results[c]["yt"]
        for t, (r0, r1) in enumerate(ROWS[g]):
            y[b, r0:r1, :] = ytc[t].reshape(1024, 512).T
    return y


# revision 12
# speedup vs baseline: 1.0543x; 1.0464x over previous
"""Trainium2 Bass kernel for the n-gram induction-head + dual-linear problem.

  y = induction_head(input_ids, x) @ W0.T + b0 + x @ W1.T + b1
  (B=4, S=2048, D=1024, ngram=2, shift_step=1)

Mask algebra (equivalent to the reference):
  maskT[j, i] = (C[i] == A[j]) & (i > j)
    A[j] = 512*ids[j-2] + ids[j-1]   (sentinels ids[-2]=-11, ids[-1]=-13)
    C[i] = 512*ids[i-1] + ids[i]
  h0[i,:] = (sum_j maskT[j,i] * x[j,:]) / max(sum_j maskT[j,i], 1)

Distribution: 8 cores = 4 batches x 2 row-groups. Group 0 takes rows
[1536:2048]+[0:512], group 1 takes [1024:1536]+[512:1024] of its batch, which
balances the causal-triangle matmul work exactly (20 live j-chunk passes per
core). The whole computation runs in a transposed dataflow (h0^T, y^T) so no
on-chip transposes are needed; the host un-transposes the per-core outputs.

The single SPMD program is identical on all cores; all per-core variation
(row ranges, ids, triangle thresholds) enters via input data.
"""

import os
from contextlib import ExitStack

import numpy as np

import concourse.bass as bass  # noqa: F401  (engine types referenced via nc)
import concourse.mybir as mybir
import concourse.tile as tile
from concourse import bacc
from concourse.bass_utils import run_bass_kernel_spmd

F32 = mybir.dt.float32
S = 2048
D = 1024
NSLOTS = [16, 8]
ROWS = {
    0: [(1536, 2048), (0, 512)],
    1: [(1024, 1536), (512, 1024)],
}


def _tri_slots(t):
    return range(8, 16) if t == 0 else range(0, 8)


def _build_kernel(tc, io, mm_dt):
    nc = tc.nc
    op = mybir.AluOpType
    xf, xt, w0t, w1t, yt = (
        io["xf"], io["xt"], io["w0t"], io["w1t"], io["yt"],
    )

    with ExitStack() as ctx:
        consts = ctx.enter_context(tc.tile_pool(name="consts", bufs=1))
        maskp = ctx.enter_context(tc.tile_pool(name="maskp", bufs=24))
        trip = ctx.enter_context(tc.tile_pool(name="trip", bufs=2))
        bcast = ctx.enter_context(tc.tile_pool(name="bcast", bufs=2))
        h0p = ctx.enter_context(tc.tile_pool(name="h0p", bufs=2))
        xtp = ctx.enter_context(tc.tile_pool(name="xtp", bufs=2))
        youtp = ctx.enter_context(tc.tile_pool(name="youtp", bufs=4))
        psum = ctx.enter_context(tc.tile_pool(name="psum", bufs=8, space="PSUM"))

        # ---- tiles ----
        ones = consts.tile([128, 128], mm_dt)
        smalls = consts.tile([128, 56], F32)
        d0 = smalls[:, 0:16]
        d1 = smalls[:, 16:32]
        jv = smalls[:, 32:48]
        bcols = smalls[:, 48:56]
        a16 = consts.tile([128, 16], F32)
        xall = consts.tile([128, 16 * 1024], mm_dt)
        w0 = consts.tile([128, 8 * 1024], mm_dt)
        w1 = consts.tile([128, 8 * 1024], mm_dt)
        cbs, ivbs, e0bs, mrows = {}, {}, {}, {}
        for t in range(2):
            mr = bcast.tile(
                [128, 3 * 512], mybir.dt.float16, tag="mr", name=f"mr{t}"
            )
            mrows[t] = mr
            e0bs[t] = mr[:, 512:1024]
            ivbs[t] = mr[:, 1024:1536]
            cbs[t] = bcast.tile([128, 512], F32, tag="cbf", name=f"cbf{t}")

        wtile = consts.tile([128, 512], mm_dt)

        # ---- DMA issue order (sync HWDGE queue): critical path first ----
        nc.sync.dma_start(ones[:], io["onesd"][:])
        nc.sync.dma_start(smalls[:], io["smalls"][:])
        nc.sync.dma_start(mrows[0][:], io["mrowsd"][0])
        nc.sync.dma_start(xall[:, 0:1024], xf[0])
        nc.sync.dma_start(xall[:, 1024:2048], xf[1])
        for s_ in range(2, 16):
            nc.sync.dma_start(xall[:, s_ * 1024 : (s_ + 1) * 1024], xf[s_])
        nc.sync.dma_start(mrows[1][:], io["mrowsd"][1])
        for kc in range(8):
            nc.sync.dma_start(w0[:, kc * 1024 : (kc + 1) * 1024], w0t[kc])
            nc.sync.dma_start(w1[:, kc * 1024 : (kc + 1) * 1024], w1t[kc])

        # ---- compute ----
        # PE warm-up burst: ~5us of matmuls on constant data during the
        # initial DMA window, so HAM un-throttles before real work arrives.
        nc.vector.memset(wtile[:], 0.0)
        wps = psum.tile([128, 512], F32, tag="ps", name="warm")
        for i in range(12):
            nc.tensor.matmul(
                wps[:], ones[:], wtile[:], start=(i == 0), stop=(i == 11)
            )
        nc.vector.scalar_tensor_tensor(
            a16[:], d0[:], 512.0, d1[:], op.mult, op.add
        )

        masks = {0: [], 1: []}
        h0s = {}

        def build_masks(t):
            cb, ivb = cbs[t], ivbs[t]
            nc.vector.scalar_tensor_tensor(
                cb[:], mrows[t][:, 0:512], 512.0, e0bs[t][:], op.mult, op.add
            )
            tri = set(_tri_slots(t))
            for s in range(NSLOTS[t]):
                m = maskp.tile([128, 512], mm_dt, tag="m", name=f"m{t}_{s}")
                if s in tri:
                    tr = trip.tile([128, 512], mm_dt, tag="tr", name=f"tr{t}_{s}")
                    nc.vector.tensor_scalar(
                        tr[:], ivb[:], jv[:, s : s + 1], None, op.is_gt
                    )
                    nc.vector.scalar_tensor_tensor(
                        m[:], cb[:], a16[:, s : s + 1], tr[:], op.is_equal, op.mult
                    )
                else:
                    nc.vector.tensor_scalar(
                        m[:], cb[:], a16[:, s : s + 1], None, op.is_equal
                    )
                masks[t].append(m)

        def attention(t):
            att = [
                psum.tile([128, 512], F32, tag="ps", name=f"att{t}_{dc}")
                for dc in range(8)
            ]
            for s in range(NSLOTS[t]):
                for dc in range(8):
                    nc.tensor.matmul(
                        att[dc][:],
                        xall[:, s * 1024 + dc * 128 : s * 1024 + dc * 128 + 128],
                        masks[t][s][:],
                        start=(s == 0),
                        stop=(s == NSLOTS[t] - 1),
                    )
            # raw (unnormalized) h0 evicted on the Scalar engine
            h0 = h0p.tile([128, 8 * 512], mm_dt, tag="h0", name=f"h0_{t}")
            for dc in range(8):
                nc.scalar.copy(h0[:, dc * 512 : (dc + 1) * 512], att[dc][:])
            h0s[t] = h0

        def denom(t):
            macc = bcast.tile(
                [128, 512], mm_dt, tag="macc", name=f"macc{t}"
            )
            nc.vector.tensor_tensor(
                macc[:], masks[t][0][:], masks[t][1][:], op.add
            )
            for s in range(2, NSLOTS[t]):
                nc.vector.tensor_tensor(macc[:], macc[:], masks[t][s][:], op.add)
            dn = psum.tile([128, 512], F32, tag="ps", name=f"dn{t}")
            nc.tensor.matmul(dn[:], ones[:], macc[:], start=True, stop=True)
            dcl = bcast.tile([128, 512], F32, tag="dcl", name=f"dcl{t}")
            nc.vector.tensor_scalar(dcl[:], dn[:], 1.0, None, op.max)
            rec = bcast.tile([128, 512], F32, tag="rec", name=f"rec{t}")
            nc.vector.reciprocal(rec[:], dcl[:])
            h0 = h0s[t]
            for dc in range(8):
                nc.vector.tensor_tensor(
                    h0[:, dc * 512 : (dc + 1) * 512],
                    h0[:, dc * 512 : (dc + 1) * 512],
                    rec[:],
                    op.mult,
                )

        def linears(t):
            h0 = h0s[t]
            xts = xtp.tile([128, 8 * 512], mm_dt, tag="xts", name=f"xts{t}")
            for kc in range(8):
                nc.sync.dma_start(xts[:, kc * 512 : (kc + 1) * 512], xt[t, kc])
            for oc in range(8):
                yps = psum.tile([128, 512], F32, tag="ps", name=f"yps{t}_{oc}")
                for kc in range(8):
                    nc.tensor.matmul(
                        yps[:],
                        w0[:, kc * 1024 + oc * 128 : kc * 1024 + oc * 128 + 128],
                        h0[:, kc * 512 : (kc + 1) * 512],
                        start=(kc == 0),
                        stop=False,
                    )
                for kc in range(8):
                    nc.tensor.matmul(
                        yps[:],
                        w1[:, kc * 1024 + oc * 128 : kc * 1024 + oc * 128 + 128],
                        xts[:, kc * 512 : (kc + 1) * 512],
                        start=False,
                        stop=(kc == 7),
                    )
                ysb = youtp.tile([128, 512], F32, tag="ysb", name=f"ysb{t}_{oc}")
                nc.scalar.activation(
                    ysb[:],
                    yps[:],
                    mybir.ActivationFunctionType.Identity,
                    bias=bcols[:, oc : oc + 1],
                    scale=1.0,
                )
                nc.sync.dma_start(yt[t, oc], ysb[:])

        build_masks(0)
        attention(0)
        build_masks(1)
        denom(0)
        attention(1)
        denom(1)
        linears(0)
        linears(1)


def _build_nc(mm_dt):
    nc = bacc.Bacc("TRN2", target_bir_lowering=False, debug=False)
    io = {
        "xf": nc.dram_tensor("xf", [16, 128, 1024], mm_dt, kind="ExternalInput"),
        "onesd": nc.dram_tensor("onesd", [128, 128], mm_dt, kind="ExternalInput"),
        "xt": nc.dram_tensor("xt", [2, 8, 128, 512], mm_dt, kind="ExternalInput"),
        "w0t": nc.dram_tensor("w0t", [8, 128, 1024], mm_dt, kind="ExternalInput"),
        "w1t": nc.dram_tensor("w1t", [8, 128, 1024], mm_dt, kind="ExternalInput"),
        "smalls": nc.dram_tensor("smalls", [128, 56], F32, kind="ExternalInput"),
        "mrowsd": nc.dram_tensor(
            "mrowsd", [2, 128, 3 * 512], mybir.dt.float16, kind="ExternalInput"
        ),
        "yt": nc.dram_tensor("yt", [2, 8, 128, 512], F32, kind="ExternalOutput"),
    }
    with tile.TileContext(nc) as tc:
        _build_kernel(tc, {k: v.ap() for k, v in io.items()}, mm_dt)
    nc.compile()
    return nc


def _make_core_inputs(c, x, ids_i, W0, b0, W1, b1, np_mm=np.float32):
    b, g = c // 2, c % 2
    xb = np.ascontiguousarray(x[b], dtype=np.float32)
    idb = ids_i[b].astype(np.float32)
    P = np.empty(S + 4, np.float32)
    P[0], P[1] = -11.0, -13.0
    P[2 : S + 2] = idb
    P[S + 2 :] = -15.0
    rows = ROWS[g]

    xt = np.empty((2, 8, 128, 512), np.float32)
    mrowsd = np.empty((2, 128, 3 * 512), np.float32)
    for t, (r0, r1) in enumerate(rows):
        xt[t] = xb[r0:r1, :].T.reshape(8, 128, 512)
        mrowsd[t, :, 0:512] = P[r0 + 1 : r0 + 513][None, :]  # cb raw = ids[i-1]
        mrowsd[t, :, 512:1024] = P[r0 + 2 : r0 + 514][None, :]  # e0b = ids[i]
        mrowsd[t, :, 1024:1536] = np.arange(r0, r1, dtype=np.float32)[None, :]

    smalls = np.empty((128, 56), np.float32)
    smalls[:, 0:16] = P[0:2048].reshape(16, 128).T
    smalls[:, 16:32] = P[1:2049].reshape(16, 128).T
    smalls[:, 32:48] = (
        np.arange(16)[None, :] * 128 + np.arange(128)[:, None]
    ).astype(np.float32)
    smalls[:, 48:56] = (
        (b0.astype(np.float32) + b1.astype(np.float32)).reshape(8, 128).T
    )
    return {
        "xf": xb.reshape(16, 128, 1024).astype(np_mm),
        "onesd": np.ones((128, 128), np_mm),
        "xt": xt.astype(np_mm),
        "w0t": np.ascontiguousarray(W0.T.astype(np.float32)).reshape(8, 128, 1024).astype(np_mm),
        "w1t": np.ascontiguousarray(W1.T.astype(np.float32)).reshape(8, 128, 1024).astype(np_mm),
        "smalls": smalls,
        "mrowsd": mrowsd.astype(np.float16),
    }


_NC_CACHE = {}


def kernel(**inputs):
    x = np.asarray(inputs["x"])
    ids = np.asarray(inputs["input_ids"])
    W0 = np.asarray(inputs["W0"])
    b0 = np.asarray(inputs["b0"])
    W1 = np.asarray(inputs["W1"])
    b1 = np.asarray(inputs["b1"])

    sel = os.environ.get("NGRAM_MM_DT", "bf16")
    mm_dt = {
        "f32": mybir.dt.float32,
        "f32r": mybir.dt.float32r,
        "bf16": mybir.dt.bfloat16,
    }[sel]
    key = str(mm_dt)
    if key not in _NC_CACHE:
        _NC_CACHE[key] = _build_nc(mm_dt)
    nc = _NC_CACHE[key]

    import ml_dtypes

    np_mm = (
        ml_dtypes.bfloat16 if mm_dt == mybir.dt.bfloat16 else np.float32
    )
    in_maps = [
        _make_core_inputs(c, x, ids, W0, b0, W1, b1, np_mm) for c in range(8)
    ]
    core_ids = list(range(8))
    trace = os.environ.get("NGRAM_TRACE", "") == "1"
    res = run_bass_kernel_spmd(nc, in_maps, core_ids, trace=trace)
    if trace and res.exec_time_ns is not None:
        print(f"HW exec time: {res.exec_time_ns} ns")
        kernel.last_exec_time_ns = res.exec_time_ns

    y = np.empty((4, S, D), np.float32)
    for c in range(8):
        b, g = c // 2, c % 2
        ytc = res.results[c]["yt"]
        for t, (r0, r1) in enumerate(ROWS[g]):
            y[b, r0:r1, :] = ytc[t].reshape(1024, 512).T
    return y
